# revision 27
# baseline (speedup 1.0000x reference)
"""GPT-NeoX attention (s=2048, b=1, h=2048, nh=16, hd=128, rot=32) on 8 NeuronCores.

Sharding: tensor-parallel over heads (2 heads per core). Each core computes
QKV^T for its heads from a host-pretransposed bf16 hidden, runs causal
attention in transposed-score layout (scores^T[j,i] so softmax sums become
matmuls), then a row-parallel slice of the dense projection. The 8 bf16
partials are summed on host (all-reduce equivalent).

Engine budget (per core): PE does all matmuls (~145us); Act does PSUM
evictions + exp; Pool does softmax accumulation + half of RoPE + dense
evictions; DVE does masks, RoPE, normalize. Output DMA rides the Act HWDGE
queue; inputs ride SP. Dense for chunk i is emitted during chunk i+1 to hide
the softmax-normalize latency and spread the output DMA.
"""

import math
import numpy as np
import ml_dtypes

S = 2048
HID = 2048
NH = 16
D = 128
ROT = 32
NCORES = 8
HPC = 2  # heads per core
CHUNK = 512
NKT = HID // 128  # 16 contraction tiles
NCH = S // CHUNK  # 4 i-chunks
NST = S // 128    # 16 s-tiles
NORM = 1.0 / math.sqrt(D)
MASK_NEG = -30000.0

BF16 = ml_dtypes.bfloat16

_cache = {}


def _build_program():
    from concourse import bass, bacc, tile
    from concourse.bass import mybir

    f32 = mybir.dt.float32
    bf16 = mybir.dt.bfloat16
    Exp = mybir.ActivationFunctionType.Exp
    Ident = mybir.ActivationFunctionType.Identity
    ADD = mybir.AluOpType.add
    MULT = mybir.AluOpType.mult

    nc = bacc.Bacc()

    ht_d = nc.dram_tensor("ht", [HID, S], bf16, kind="ExternalInput")
    wq_d = nc.dram_tensor("wq", [HID, HPC * D], bf16, kind="ExternalInput")
    wk_d = nc.dram_tensor("wk", [HID, HPC * D], bf16, kind="ExternalInput")
    wv_d = nc.dram_tensor("wv", [HID, HPC * D], bf16, kind="ExternalInput")
    wd_d = nc.dram_tensor("wd", [HPC * D, HID], bf16, kind="ExternalInput")
    cos_d = nc.dram_tensor("cosT", [ROT, S], bf16, kind="ExternalInput")
    sin_d = nc.dram_tensor("sinT", [ROT, S], bf16, kind="ExternalInput")
    rp_d = nc.dram_tensor("rperm", [ROT, ROT], bf16, kind="ExternalInput")
    mask_d = nc.dram_tensor("maskbias", [128, 128], f32, kind="ExternalInput")
    bqk_d = nc.dram_tensor("bqk", [128, 4], f32, kind="ExternalInput")
    bvb_d = nc.dram_tensor("bvb", [128, HPC * D], f32, kind="ExternalInput")
    out_d = nc.dram_tensor("partial", [S, HID], bf16, kind="ExternalOutput")

    ht_r = ht_d.rearrange("(k p) s -> p k s", p=128)
    wq_r = wq_d.rearrange("(k p) m -> p k m", p=128)
    wk_r = wk_d.rearrange("(k p) m -> p k m", p=128)
    wv_r = wv_d.rearrange("(k p) m -> p k m", p=128)
    wd_r = wd_d.rearrange("(c p) o -> p c o", p=128)

    with tile.TileContext(nc) as tc:
        with (
            tc.tile_pool(name="persist", bufs=1) as pp,
            tc.tile_pool(name="probs", bufs=4) as prp,
            tc.tile_pool(name="small", bufs=2) as smp,
            tc.tile_pool(name="rotp", bufs=2) as rop,
            tc.tile_pool(name="recb", bufs=2) as rcb,
            tc.tile_pool(name="ctxp", bufs=2) as ctp,
            tc.tile_pool(name="stage", bufs=3) as stp,
            tc.tile_pool(name="ps_a", bufs=2, space="PSUM") as pa,
            tc.tile_pool(name="ps_b", bufs=3, space="PSUM") as pb,
            tc.tile_pool(name="ps_s", bufs=2, space="PSUM") as ps_s,
            tc.tile_pool(name="ps_d", bufs=1, space="PSUM") as pdp,
        ):
            # ---- persistent SBUF tiles ----
            ht = pp.tile([128, NKT, S], bf16, tag="ht")
            wq = pp.tile([128, NKT, HPC * D], bf16, tag="wq")
            wk = pp.tile([128, NKT, HPC * D], bf16, tag="wk")
            wv = pp.tile([128, NKT, HPC * D], bf16, tag="wv")
            wd = pp.tile([128, HPC, HID], bf16, tag="wd")
            cosT = pp.tile([ROT, S], bf16, tag="cos")
            sinT = pp.tile([ROT, S], bf16, tag="sin")
            rperm = pp.tile([ROT, ROT], bf16, tag="rperm")
            maskb = pp.tile([128, 128], f32, tag="mask")
            bqk = pp.tile([128, 4], f32, tag="bqk")
            bvb = pp.tile([128, HPC * D], f32, tag="bvb")
            qT = [pp.tile([128, S], bf16, tag=f"qT{h}", name=f"qT{h}") for h in range(HPC)]
            kT = [pp.tile([128, S], bf16, tag=f"kT{h}", name=f"kT{h}") for h in range(HPC)]
            vn = [pp.tile([128, NST, D], bf16, tag=f"vn{h}", name=f"vn{h}") for h in range(HPC)]
            onesm = pp.tile([128, 128], bf16, tag="onesm")

            nc.vector.memset(onesm[:], 1.0)

            # ---- input DMAs: ht/wd/outputs ride the SP queue; weights and
            # consts ride the Act HWDGE queue (idle at startup) so the first
            # projection chain can stream as ht tiles land ----
            nc.sync.dma_start(bqk[:], bqk_d[:])
            nc.sync.dma_start(wq[:], wq_r[:])
            for k in range(NKT):
                nc.sync.dma_start(ht[:, k, 0:CHUNK], ht_r[:, k, 0:CHUNK])
            nc.sync.dma_start(wk[:], wk_r[:])
            nc.sync.dma_start(rperm[:], rp_d[:])
            nc.sync.dma_start(cosT[:], cos_d[:])
            nc.sync.dma_start(sinT[:], sin_d[:])
            nc.sync.dma_start(wv[:], wv_r[:])
            nc.sync.dma_start(maskb[:], mask_d[:])
            nc.sync.dma_start(bvb[:], bvb_d[:])
            for k in range(NKT):
                nc.sync.dma_start(ht[:, k, CHUNK:2 * CHUNK], ht_r[:, k, CHUNK:2 * CHUNK])
            nc.sync.dma_start(wd[:], wd_r[:])
            for ci in (2, 3):
                sl = slice(ci * CHUNK, (ci + 1) * CHUNK)
                for k in range(NKT):
                    nc.sync.dma_start(ht[:, k, sl], ht_r[:, k, sl])

            ctxc = [[None, None] for _ in range(NCH)]  # [ci][h] -> ctx tile

            def dense_st(ci, st4, dve_only=False):
                st = 4 * ci + st4
                ssl = slice(st4 * 128, (st4 + 1) * 128)
                stg = stp.tile([128, NCH, CHUNK], bf16, tag="stg", name="stg")
                for oc in range(NCH):
                    osl = slice(oc * CHUNK, (oc + 1) * CHUNK)
                    po = pb.tile([128, CHUNK], f32, tag="b", name="po")
                    nc.tensor.matmul(po[:], ctxc[ci][0][:, ssl], wd[:, 0, osl],
                                     start=True, stop=False)
                    nc.tensor.matmul(po[:], ctxc[ci][1][:, ssl], wd[:, 1, osl],
                                     start=False, stop=True)
                    if oc <= 1 and not dve_only:
                        nc.scalar.activation(stg[:, oc, :], po[:], Ident)
                    else:
                        nc.vector.tensor_copy(stg[:, oc, :], po[:])
                nc.sync.dma_start(out_d[st * 128:(st + 1) * 128, :], stg[:])

            def dense(ci):
                for st4 in range(4):
                    dense_st(ci, st4)

            for ci in range(NCH):
                isl = slice(ci * CHUNK, (ci + 1) * CHUNK)

                # ---- q/k projection (transposed layout) + bias ----
                for (w, dstT, bcol) in ((wq, qT, 0), (wk, kT, 2)):
                    for h in range(HPC):
                        ps = pa.tile([128, CHUNK], f32, tag="a", name="ps_qk")
                        for k in range(NKT):
                            nc.tensor.matmul(
                                ps[:], w[:, k, h * D:(h + 1) * D], ht[:, k, isl],
                                start=(k == 0), stop=(k == NKT - 1),
                            )
                        nc.scalar.activation(
                            dstT[h][:, isl], ps[:], Ident,
                            bias=bqk[:, bcol + h:bcol + h + 1],
                        )

                # ---- RoPE on rot rows: t[0:32] = t*cos + (R @ t)*sin ----
                # (Pool cannot touch PSUM, so the psr read is on DVE; the
                # SBUF-only in-place multiply/add go to Pool.)
                for ti, t in enumerate((qT[0], qT[1], kT[0], kT[1])):
                    psr = ps_s.tile([ROT, CHUNK], f32, tag="s", name="psr")
                    nc.tensor.matmul(psr[:], rperm[:], t[0:ROT, isl],
                                     start=True, stop=True)
                    ru = rop.tile([ROT, CHUNK], bf16, tag=f"ru{ti % 2}", name="ru")
                    nc.vector.tensor_tensor(ru[:], psr[:], sinT[:, isl], MULT)
                    nc.vector.tensor_tensor(t[0:ROT, isl], t[0:ROT, isl],
                                            cosT[:, isl], MULT)
                    nc.vector.tensor_tensor(t[0:ROT, isl], t[0:ROT, isl], ru[:], ADD)

                # ---- V projection (natural layout), bias folded into chain ----
                for st4 in range(4):
                    st = 4 * ci + st4
                    ps = pb.tile([128, HPC * D], f32, tag="b", name="ps_v")
                    for k in range(NKT):
                        nc.tensor.matmul(
                            ps[:], ht[:, k, st * 128:(st + 1) * 128], wv[:, k, :],
                            start=(k == 0), stop=(k == NKT - 1),
                        )
                    for h in range(HPC):
                        nc.vector.tensor_tensor(
                            vn[h][:, st, :], ps[:, h * D:(h + 1) * D],
                            bvb[:, h * D:(h + 1) * D], ADD)

                # ---- dense of the previous chunk (hides normalize latency);
                # for the last chunk it is interleaved into the attention
                # j-loop below so PE fills its exp-wait gaps ----
                if ci in (1, 2):
                    dense(ci - 1)
                fillers = (
                    [lambda s=s: dense_st(ci - 1, s, dve_only=True) for s in range(4)]
                    if ci == NCH - 1 else []
                )
                tiles_done = 0

                # ---- causal attention, transposed-score layout ----
                nt = 4 * ci + 4
                pctxs, recrs = [], []
                for h in range(HPC):
                    pctx = pa.tile([128, CHUNK], f32, tag="a", name="pctx")
                    pden = pdp.tile([128, CHUNK], f32, tag="d", name="pden")
                    for t in range(nt):
                        off = max((t - 4 * ci) * 128, 0)
                        cols = slice(off, CHUNK)
                        pss = ps_s.tile([128, CHUNK], f32, tag="s", name="pss")
                        nc.tensor.matmul(
                            pss[:, cols], kT[h][:, t * 128:(t + 1) * 128],
                            qT[h][:, ci * CHUNK + off:(ci + 1) * CHUNK],
                            start=True, stop=True,
                        )
                        if t >= 4 * ci:
                            nc.vector.tensor_tensor(
                                pss[:, off:off + 128], pss[:, off:off + 128],
                                maskb[:], ADD,
                            )
                        probs = prp.tile([128, CHUNK], bf16, tag="probs", name="probs")
                        nc.scalar.activation(probs[:, cols], pss[:, cols], Exp,
                                             scale=NORM)
                        # softmax denominator: third interleaved PSUM
                        # accumulation chain on the PE. lhsT is an all-ones
                        # [128,128] matrix, so every output row carries the
                        # column sums -- the partition broadcast comes free.
                        nc.tensor.matmul(
                            pden[:, cols], onesm[:], probs[:, cols],
                            start=(t == 0), stop=(t == nt - 1),
                            skip_group_check=True,
                        )
                        nc.tensor.matmul(
                            pctx[:, cols], vn[h][:, t, :], probs[:, cols],
                            start=(t == 0), stop=(t == nt - 1),
                            skip_group_check=True,
                        )
                        tiles_done += 1
                        if fillers and tiles_done % 6 == 0:
                            fillers.pop(0)()
                    rec = rcb.tile([128, CHUNK], bf16, tag="rec", name="rec")
                    with nc.allow_low_precision(reason="1/den in bf16: ~0.4% rel, "
                                                "well under the 2e-2 budget"):
                        nc.vector.reciprocal(rec[:], pden[:])
                    pctxs.append(pctx)
                    recrs.append(rec)
                for h in range(HPC):
                    ctx = ctp.tile([128, CHUNK], bf16, tag=f"ctx{h}", name=f"ctx{h}")
                    nc.vector.tensor_tensor(ctx[:], pctxs[h], recrs[h], MULT)
                    ctxc[ci][h] = ctx

            dense(NCH - 1)

    nc.compile()
    return nc


def _prep_inputs(hidden_states, W_qkv, b_qkv, W_dense, b_dense):
    hid = np.asarray(hidden_states).reshape(S, HID)
    hT = np.ascontiguousarray(hid.T).astype(BF16)

    inv_freq = 1.0 / (10000.0 ** (np.arange(0, ROT, 2, dtype=np.float64) / ROT))
    t = np.arange(S, dtype=np.float64)
    freqs = np.outer(t, inv_freq)                      # [s, rot/2]
    emb = np.concatenate([freqs, freqs], axis=1)       # [s, rot]
    cosT = np.ascontiguousarray(np.cos(emb).T).astype(BF16)
    sinT = np.ascontiguousarray(np.sin(emb).T).astype(BF16)

    # rotate-half permutation: out[r] = sum_c rperm[c, r] * x[c]
    rp = np.zeros((ROT, ROT), np.float32)
    half = ROT // 2
    for c in range(half):
        rp[c, c + half] = 1.0
    for c in range(half, ROT):
        rp[c, c - half] = -1.0
    rp = rp.astype(BF16)

    maskb = np.where(
        np.arange(128)[:, None] > np.arange(128)[None, :], MASK_NEG, 0.0
    ).astype(np.float32) / NORM  # pre-divide: exp applies scale=NORM

    in_maps = []
    for c in range(NCORES):
        heads = [HPC * c, HPC * c + 1]
        wqc = np.concatenate([W_qkv[:, n * 384: n * 384 + 128] for n in heads], 1)
        wkc = np.concatenate([W_qkv[:, n * 384 + 128: n * 384 + 256] for n in heads], 1)
        wvc = np.concatenate([W_qkv[:, n * 384 + 256: n * 384 + 384] for n in heads], 1)
        bq = np.stack([b_qkv[n * 384: n * 384 + 128] for n in heads], 1)
        bk = np.stack([b_qkv[n * 384 + 128: n * 384 + 256] for n in heads], 1)
        bv = np.concatenate([b_qkv[n * 384 + 256: n * 384 + 384] for n in heads])
        bqk = np.concatenate([bq, bk], axis=1).astype(np.float32)  # [128,4] q0 q1 k0 k1
        bvb = np.ascontiguousarray(np.broadcast_to(bv, (128, HPC * D))).astype(np.float32)
        wdd = W_dense[c * HPC * D:(c + 1) * HPC * D, :]
        in_maps.append({
            "ht": hT,
            "wq": np.ascontiguousarray(wqc).astype(BF16),
            "wk": np.ascontiguousarray(wkc).astype(BF16),
            "wv": np.ascontiguousarray(wvc).astype(BF16),
            "wd": np.ascontiguousarray(wdd).astype(BF16),
            "cosT": cosT,
            "sinT": sinT,
            "rperm": rp,
            "maskbias": maskb,
            "bqk": np.ascontiguousarray(bqk),
            "bvb": bvb,
        })
    return in_maps


def _finish(results, inputs):
    partial = np.zeros((S, HID), np.float32)
    for r in results:
        partial += np.asarray(r["partial"], dtype=np.float32)
    out = partial + np.asarray(inputs["b_dense"], dtype=np.float32)[None, :]
    return out.reshape(S, 1, HID)


def _run(inputs, trace=False):
    from concourse.bass_utils import run_bass_kernel_spmd

    if "nc" not in _cache:
        _cache["nc"] = _build_program()
    nc = _cache["nc"]
    in_maps = _prep_inputs(
        inputs["hidden_states"], inputs["W_qkv"], inputs["b_qkv"],
        inputs["W_dense"], inputs["b_dense"],
    )
    res = run_bass_kernel_spmd(nc, in_maps, list(range(NCORES)), trace=trace)
    return _finish(res.results, inputs), res


def kernel(**inputs):
    out, _ = _run(inputs, trace=False)
    return out
